# revision 4
# baseline (speedup 1.0000x reference)
"""Trainium2 Bass kernel for nn_BatchSampler (sampling / expanded border matrix).

Decomposition: the lexsorted "expanded border matrix" output (L ~ 6.55M rows)
is 99.9% a periodic 50-bin border pattern over 130816 node pairs, with ~5.3K
sparse edge-event insertions. We split the P=130816 pairs into 8 cores x 511
rows of 32 pairs; each row is an independent segment of 1600 border slots plus
its inserted edges (max D per row, data-dependent, ~7). Each core's 4 tiles of
[128 rows x W slots] are computed on-device with static access patterns:
  - nins (running insertion count per slot) via D fused compare-add ops
  - border times / pair ids via a D-level shift cascade (copy_predicated)
    reading a prebuilt periodic pattern and per-row pair tables
  - edge time/state injection via masked per-slot overwrites
  - expanded_states via a hardware prefix scan (forward fill with resets)
  - dt via shifted difference with wrap fix-up
Rows are written padded; the host drops the per-row padding and concatenates.
The host precomputes only O(P + E) metadata (tables, per-row edge slots) and
the batch_nodes Gumbel top-k sample (replicated, jax CPU PRNG as reference).
"""
import os
import sys

for _p in ("/opt/trn_rl_repo", "/root/.axon_site/_ro/trn_rl_repo"):
    if os.path.isdir(_p) and _p not in sys.path:
        sys.path.append(_p)

import numpy as np

N_NODES = 5000
BATCH = 512
N_BINS = 50
N_CORES = 8
PAIRS_PER_ROW = 32
W_BORDER = PAIRS_PER_ROW * N_BINS  # 1600


# ---------------------------------------------------------------------------
# host-side decomposition
# ---------------------------------------------------------------------------

def _batch_nodes_host():
    import jax
    import jax.numpy as jnp

    with jax.default_device(jax.local_devices(backend="cpu")[0]):
        key = jax.random.key(19)
        n = N_NODES
        w = jnp.arange(n, dtype=jnp.float32)
        logw = jnp.where(w > 0, jnp.log(jnp.maximum(w, 1.0)), -jnp.inf)
        g = jax.random.gumbel(key, (n,))
        bn = jnp.sort(jax.lax.top_k(logw + g, BATCH)[1])
        return np.asarray(bn, dtype=np.int32)


def _decompose(edges, edge_times, edge_states, bin_bounds):
    n, B, bins = N_NODES, BATCH, N_BINS
    edges = np.asarray(edges)
    edge_times = np.asarray(edge_times, dtype=np.float32)
    edge_states = np.asarray(edge_states)
    bin_bounds = np.asarray(bin_bounds, dtype=np.float32)

    bn = _batch_nodes_host()

    ti, tj = np.triu_indices(B, k=1)
    i_table = bn[ti].astype(np.int32)
    j_table = bn[tj].astype(np.int32)
    P = i_table.size

    pos_in_batch = np.full(n, -1, np.int64)
    pos_in_batch[bn] = np.arange(B)
    e0 = np.asarray(edges[0], dtype=np.int64)
    e1 = np.asarray(edges[1], dtype=np.int64)
    u = pos_in_batch[e0]
    v = pos_in_batch[e1]
    selmask = (u >= 0) & (v >= 0)
    su, sv = u[selmask], v[selmask]
    st = edge_times[selmask]
    ss = edge_states[selmask].astype(np.float32)
    r = su * (B - 1) - (su * (su - 1)) // 2 + (sv - su - 1)

    order = np.lexsort((ss, st, r))
    r, st, ss = r[order], st[order], ss[order]
    Esel = r.size

    jm = np.searchsorted(bin_bounds[:bins], st, side="right")

    pairs_per_core = P // N_CORES
    rows_per_core = pairs_per_core // PAIRS_PER_ROW + 1  # 512 (last is pad)
    nrows = rows_per_core * N_CORES

    core = r // pairs_per_core
    r_in_core = r % pairs_per_core
    row = core * rows_per_core + r_in_core // PAIRS_PER_ROW

    border_off = 50 * (r_in_core % PAIRS_PER_ROW) + jm
    _, row_start_idx, row_counts = np.unique(row, return_index=True, return_counts=True)
    rank_in_row = np.arange(Esel) - np.repeat(row_start_idx, row_counts)
    epos = (border_off + rank_in_row).astype(np.int64)

    e_per_row = np.zeros(nrows, np.int64)
    np.add.at(e_per_row, row, 1)
    D = max(int(e_per_row.max()) if Esel else 0, 1)

    EPOS = np.full((nrows, D), 30000, np.float32)
    ETIME = np.zeros((nrows, D), np.float32)
    ESTATE = np.zeros((nrows, D), np.float32)
    EPOS[row, rank_in_row] = epos
    ETIME[row, rank_in_row] = st
    ESTATE[row, rank_in_row] = ss

    npairs_row = np.full(nrows, PAIRS_PER_ROW, np.int64)
    npairs_row[rows_per_core - 1 :: rows_per_core] = 0
    lam = npairs_row * 50 + e_per_row

    TSI = np.zeros((nrows, 33), np.float32)
    TSJ = np.zeros((nrows, 33), np.float32)
    valid = npairs_row > 0
    row_ids = np.arange(nrows)
    first_pair = (row_ids // rows_per_core) * pairs_per_core + (
        row_ids % rows_per_core
    ) * PAIRS_PER_ROW
    idx = np.minimum(first_pair[valid, None] + np.arange(32)[None, :], P - 1)
    TSI[valid, :32] = i_table[idx].astype(np.float32)
    TSJ[valid, :32] = j_table[idx].astype(np.float32)

    WO = W_BORDER + D  # max valid row length
    PATLEN = W_BORDER + 2 * bins  # covers s in [0, WO] for D <= 50
    PAT = np.tile(bin_bounds[:bins], PAIRS_PER_ROW + 2)[:PATLEN].astype(np.float32)

    return dict(
        bn=bn, D=D, WO=WO, PATLEN=PATLEN, PAT=PAT,
        nrows=nrows, rows_per_core=rows_per_core,
        EPOS=EPOS, ETIME=ETIME, ESTATE=ESTATE, lam=lam,
        TSI=TSI, TSJ=TSJ,
        bb0=float(bin_bounds[0]), bb_last=float(bin_bounds[bins]),
        L=int(lam.sum()),
    )


# ---------------------------------------------------------------------------
# device kernel
# ---------------------------------------------------------------------------

_KERNEL_CACHE = {}


def _patch_tile_drain():
    """This walrus build rejects >1 sync wait per ctrl instruction; split the
    TileContext final drain's waits across multiple drains."""
    import concourse.mybir as mybir
    from concourse import tile
    from concourse.vector_clock import ScopedClock

    if getattr(tile.TileContext, "_drain_patched", False):
        return

    def _drain_and_barrier(self, tick_clock, wait_clock):
        drain_inst = self.nc.sync.drain()
        wait_clock.add_sem_waits(
            drain_inst.ins, ScopedClock({None: tick_clock.global_clock})
        )
        si = drain_inst.ins.sync_info
        waits = list(si.on_wait or [])
        if len(waits) > 1:
            si.on_wait = waits[:1]
            for i in range(1, len(waits)):
                d2 = self.nc.sync.drain()
                d2.ins.sync_info = mybir.SyncInfo(
                    on_wait=waits[i : i + 1], on_update=[]
                )
        self.nc.all_engine_barrier()
        assert self.sems is not None
        popped = self.nc._tile_sem_poison_stack.pop()
        assert popped is self._sem_poison
        self.nc.clear_and_free_semaphores(list(self.sems.allocated().values()))
        self.nc.all_engine_barrier()

    tile.TileContext._drain_and_barrier = _drain_and_barrier

    # This walrus build also rejects >1 sync wait on regular instructions.
    # Split excess waits onto preceding NoOps on the same engine at commit.
    _orig_commit = tile.TileContext._commit_instruction
    _ctr = [0]

    def _commit_split(self, inst, lazy_reg_writes=True):
        si = inst.sync_info
        if si is not None and si.on_wait and len(si.on_wait) > 1:
            waits = list(si.on_wait)
            for w in waits[:-1]:
                _ctr[0] += 1
                nop = mybir.InstNoOp(
                    name=f"waitsplit-{_ctr[0]}",
                    sync_info=mybir.SyncInfo(on_wait=[w], on_update=[]),
                    bass_nofuse=True,
                    engine=inst.engine,
                )
                _orig_commit(self, nop, lazy_reg_writes=False)
            si.on_wait = waits[-1:]
        return _orig_commit(self, inst, lazy_reg_writes)

    tile.TileContext._commit_instruction = _commit_split
    tile.TileContext._drain_patched = True


def _build_bass(D, WO, PATLEN, bb0, bb_last, rows_per_core):
    import concourse.bass as bass
    import concourse.mybir as mybir
    from concourse import tile

    _patch_tile_drain()

    f32 = mybir.dt.float32
    i16 = mybir.dt.int16
    i32 = mybir.dt.int32
    u8 = mybir.dt.uint8
    Alu = mybir.AluOpType

    NT = rows_per_core // 128  # tiles per core (4)
    W = WO  # output row width
    WT = WO + 1  # times tile width (one extra col for dt shift)

    nc = bass.Bass("TRN2", target_bir_lowering=False, debug=False,
                   num_devices=N_CORES)

    # inputs (per core)
    EPOSt = nc.dram_tensor("EPOS", [NT, 128, D], f32, kind="ExternalInput")
    LAM1t = nc.dram_tensor("LAM1", [NT, 128, 1], f32, kind="ExternalInput")
    ETIMEt = nc.dram_tensor("ETIME", [NT, 128, D], f32, kind="ExternalInput")
    ESTATEt = nc.dram_tensor("ESTATE", [NT, 128, D], f32, kind="ExternalInput")
    TSIt = nc.dram_tensor("TSI", [NT, 128, 33], f32, kind="ExternalInput")
    TSJt = nc.dram_tensor("TSJ", [NT, 128, 33], f32, kind="ExternalInput")
    PATt = nc.dram_tensor("PAT", [128, PATLEN], f32, kind="ExternalInput")

    # outputs (per core)
    OT = nc.dram_tensor("OT", [NT, 128, W], f32, kind="ExternalOutput")
    ODT = nc.dram_tensor("ODT", [NT, 128, W], f32, kind="ExternalOutput")
    OST = nc.dram_tensor("OST", [NT, 128, W], i32, kind="ExternalOutput")
    OIE = nc.dram_tensor("OIE", [NT, 128, W], u8, kind="ExternalOutput")
    OI = nc.dram_tensor("OI", [NT, 128, W], i32, kind="ExternalOutput")
    OJ = nc.dram_tensor("OJ", [NT, 128, W], i32, kind="ExternalOutput")

    with tile.TileContext(nc) as tc:
        with (
            tc.tile_pool(name="static", bufs=1) as sp,
            tc.tile_pool(name="meta", bufs=2) as mp,
            tc.tile_pool(name="work", bufs=2) as wp,
            tc.tile_pool(name="out", bufs=2) as op,
        ):
            # static tiles
            iota = sp.tile([128, WT], i16)
            nc.gpsimd.iota(iota[:], pattern=[[1, WT]], base=0, channel_multiplier=0)
            patb = sp.tile([128, D + PATLEN], f32)
            nc.gpsimd.memset(patb[:, 0:D], 0.0)
            nc.sync.dma_start(patb[:, D:], PATt[:, :])
            zerocol = sp.tile([128, 1], f32)
            nc.gpsimd.memset(zerocol[:], 0.0)

            for t in range(NT):
                # --- load metadata ---
                epos = mp.tile([128, D], f32, tag="epos")
                nc.sync.dma_start(epos[:], EPOSt[t])
                lam1 = mp.tile([128, 1], f32, tag="lam1")
                nc.sync.dma_start(lam1[:], LAM1t[t])
                etime = mp.tile([128, D], f32, tag="etime")
                nc.sync.dma_start(etime[:], ETIMEt[t])
                estate = mp.tile([128, D], f32, tag="estate")
                nc.sync.dma_start(estate[:], ESTATEt[t])
                tsi = mp.tile([128, 33], f32, tag="tsi")
                nc.sync.dma_start(tsi[:], TSIt[t])
                tsj = mp.tile([128, 33], f32, tag="tsj")
                nc.sync.dma_start(tsj[:], TSJt[t])

                # --- nins: running count of insertions at-or-before w ---
                nins = wp.tile([128, WT], i16, tag="nins")
                nc.vector.tensor_scalar(
                    nins[:], iota[:], epos[:, 0:1], None, op0=Alu.is_ge
                )
                for d in range(1, D):
                    nc.vector.scalar_tensor_tensor(
                        nins[:], iota[:], epos[:, d : d + 1], nins[:],
                        op0=Alu.is_ge, op1=Alu.add,
                    )

                # --- ind: is-edge indicator (diff of nins) ---
                ind = wp.tile([128, W], i16, tag="ind")
                nc.gpsimd.memset(ind[:, 0:1], 0)
                nc.vector.tensor_tensor(
                    ind[:, 1:W], nins[:, 1:W], nins[:, 0 : W - 1], op=Alu.subtract
                )

                # --- base i/j: per-row pair tables expanded 50x ---
                basei = wp.tile([128, D + PATLEN], f32, tag="basei")
                nc.scalar.copy(
                    basei[:, D : D + 1650], tsi[:].broadcast_to([128, 33, 50])
                )
                basej = wp.tile([128, D + PATLEN], f32, tag="basej")
                nc.scalar.copy(
                    basej[:, D : D + 1650], tsj[:].broadcast_to([128, 33, 50])
                )

                # --- shift cascade: x[w] = base[w - nins[w]] ---
                times = op.tile([128, WT], f32, tag="times")
                nc.scalar.copy(times[:], patb[:, D : D + WT])
                iw = op.tile([128, W], f32, tag="iw")
                nc.scalar.copy(iw[:], basei[:, D : D + W])
                jw = op.tile([128, W], f32, tag="jw")
                nc.scalar.copy(jw[:], basej[:, D : D + W])
                for d in range(1, D + 1):
                    eq = wp.tile([128, WT], i16, tag="eq")
                    nc.gpsimd.tensor_scalar(
                        eq[:], nins[:], float(d), None, op0=Alu.is_equal
                    )
                    nc.vector.copy_predicated(
                        times[:], eq[:], patb[:, D - d : D - d + WT]
                    )
                    nc.vector.copy_predicated(
                        iw[:], eq[:, :W], basei[:, D - d : D - d + W]
                    )
                    nc.vector.copy_predicated(
                        jw[:], eq[:, :W], basej[:, D - d : D - d + W]
                    )

                # --- edge injection: times and estates inject ---
                inject = wp.tile([128, W], f32, tag="inject")
                nc.gpsimd.memset(inject[:], 0.0)
                for d in range(D):
                    em = wp.tile([128, W], i16, tag="em")
                    nc.gpsimd.tensor_scalar(
                        em[:], iota[:, :W], epos[:, d : d + 1], None,
                        op0=Alu.is_equal,
                    )
                    nc.vector.copy_predicated(
                        times[:, :W], em[:],
                        etime[:, d : d + 1].broadcast_to([128, W]),
                    )
                    nc.vector.copy_predicated(
                        inject[:], em[:],
                        estate[:, d : d + 1].broadcast_to([128, W]),
                    )

                # --- expanded states: forward-fill scan ---
                keep = wp.tile([128, W], f32, tag="keep")
                nc.vector.tensor_scalar(
                    keep[:], times[:, :W], bb0, None, op0=Alu.not_equal
                )
                nc.vector.copy_predicated(
                    keep[:], ind[:], zerocol[:].broadcast_to([128, W])
                )
                est = op.tile([128, W], f32, tag="est")
                nc.vector.tensor_tensor_scan(
                    est[:], keep[:], inject[:], 0.0, op0=Alu.mult, op1=Alu.add
                )

                # --- dt ---
                dt = op.tile([128, W], f32, tag="dt")
                nc.vector.tensor_tensor(
                    dt[:], times[:, 1:WT], times[:, 0:W], op=Alu.subtract
                )
                negt = wp.tile([128, W], f32, tag="negt")
                nc.scalar.mul(negt[:], times[:, :W], -1.0)
                lamm = wp.tile([128, W], i16, tag="lamm")
                nc.gpsimd.tensor_scalar(
                    lamm[:], iota[:, :W], lam1[:, 0:1], None, op0=Alu.is_equal
                )
                nc.vector.copy_predicated(dt[:], lamm[:], negt[:])
                dneg = wp.tile([128, W], f32, tag="dneg")
                nc.vector.tensor_scalar(dneg[:], dt[:], 0.0, None, op0=Alu.is_lt)
                nc.vector.scalar_tensor_tensor(
                    dt[:], dneg[:], bb_last, dt[:], op0=Alu.mult, op1=Alu.add
                )

                # --- ind as u8 for is_edge output ---
                ie8 = op.tile([128, W], u8, tag="ie8")
                nc.gpsimd.tensor_copy(ie8[:], ind[:])

                # --- stores ---
                nc.sync.dma_start(OT[t], times[:, :W])
                nc.sync.dma_start(ODT[t], dt[:])
                nc.gpsimd.dma_start(OST[t], est[:])     # f32 -> i32 cast
                nc.sync.dma_start(OIE[t], ie8[:])
                nc.gpsimd.dma_start(OI[t], iw[:])       # f32 -> i32 cast
                nc.gpsimd.dma_start(OJ[t], jw[:])       # f32 -> i32 cast

    return nc


def _get_bass(meta):
    key = (meta["D"], meta["WO"], meta["PATLEN"], meta["bb0"], meta["bb_last"],
           meta["rows_per_core"])
    if key not in _KERNEL_CACHE:
        _KERNEL_CACHE[key] = _build_bass(*key)
    return _KERNEL_CACHE[key]


def _run_device(meta, trace=False):
    from concourse.bass_utils import run_bass_kernel_spmd

    nc = _get_bass(meta)
    NT = meta["rows_per_core"] // 128
    D = meta["D"]
    nrows = meta["nrows"]
    rpc = meta["rows_per_core"]

    lam1 = (meta["lam"] - 1).astype(np.float32)

    in_maps = []
    for c in range(N_CORES):
        sl = slice(c * rpc, (c + 1) * rpc)
        in_maps.append({
            "EPOS": meta["EPOS"][sl].reshape(NT, 128, D),
            "LAM1": lam1[sl].reshape(NT, 128, 1),
            "ETIME": meta["ETIME"][sl].reshape(NT, 128, D),
            "ESTATE": meta["ESTATE"][sl].reshape(NT, 128, D),
            "TSI": meta["TSI"][sl].reshape(NT, 128, 33),
            "TSJ": meta["TSJ"][sl].reshape(NT, 128, 33),
            "PAT": np.ascontiguousarray(
                np.broadcast_to(meta["PAT"], (128, meta["PATLEN"]))),
        })

    res = run_bass_kernel_spmd(
        nc, in_maps, core_ids=list(range(N_CORES)), trace=trace
    )
    return res


def _assemble(meta, res):
    nrows = meta["nrows"]
    W = meta["WO"]
    lam = meta["lam"]
    L = meta["L"]

    def stack(name):
        return np.concatenate(
            [res.results[c][name].reshape(-1, W) for c in range(N_CORES)], axis=0
        )

    row_of = np.repeat(np.arange(nrows), lam)
    starts = np.concatenate([[0], np.cumsum(lam)[:-1]])
    w_of = np.arange(L) - np.repeat(starts, lam)
    src = row_of * W + w_of

    times = stack("OT").reshape(-1)[src]
    dt = stack("ODT").reshape(-1)[src]
    est = stack("OST").reshape(-1)[src]
    ie = stack("OIE").reshape(-1)[src].astype(bool)
    i_out = stack("OI").reshape(-1)[src]
    j_out = stack("OJ").reshape(-1)[src]

    pairs = np.stack([i_out, j_out]).astype(np.int32)
    return (meta["bn"], pairs, times.astype(np.float32),
            est.astype(np.int32), ie, dt.astype(np.float32))


def kernel(edges, edge_times, edge_states, bin_bounds, nodes_num, batch_size,
           _trace=False, _return_res=False):
    meta = _decompose(edges, edge_times, edge_states, bin_bounds)
    res = _run_device(meta, trace=_trace)
    out = _assemble(meta, res)
    if _return_res:
        return out, res, meta
    return out


# revision 5
# speedup vs baseline: 17.1029x; 17.1029x over previous
"""Trainium2 Bass kernel for nn_BatchSampler (sampling / expanded border matrix).

Decomposition: the lexsorted "expanded border matrix" output (L ~ 6.55M rows)
is a periodic 50-bin border pattern over 130816 node pairs with ~5.3K sparse
edge-event insertions. Pairs are split across 8 cores x 511 rows of 32 pairs;
each row is an independent segment of 1600 border slots plus its inserted
edges. Each core's 4 tiles of [128 rows x 1608 slots] are computed on-device:
  - nins (running insertion count per slot) via a hardware prefix scan
  - border times via exact small-int arithmetic ((w - nins) mod 50 * step)
  - pair ids via a periodicity select between two shifted views of the
    expanded pair table (i and j packed into one int32)
  - edge time/state injection via predicated copies
  - expanded_states via a prefix scan (forward fill with resets at pair
    starts), dt via shifted difference with wrap fix-up (the phantom slot
    after each row's last element computes to exactly 0.0, making the
    wrap at row end automatic)
Rows are written padded; the host drops per-row padding and concatenates.
The host precomputes batch_nodes (Gumbel top-k, replicated) and the edge
selection / ordering / placement metadata.
"""
import os
import sys

for _p in ("/opt/trn_rl_repo", "/root/.axon_site/_ro/trn_rl_repo"):
    if os.path.isdir(_p) and _p not in sys.path:
        sys.path.append(_p)

import numpy as np

N_NODES = 5000
BATCH = 512
N_BINS = 50
N_CORES = 8
PAIRS_PER_ROW = 32
W_BORDER = PAIRS_PER_ROW * N_BINS  # 1600


# ---------------------------------------------------------------------------
# host-side decomposition
# ---------------------------------------------------------------------------

def _batch_nodes_host():
    import jax
    import jax.numpy as jnp

    with jax.default_device(jax.local_devices(backend="cpu")[0]):
        key = jax.random.key(19)
        n = N_NODES
        w = jnp.arange(n, dtype=jnp.float32)
        logw = jnp.where(w > 0, jnp.log(jnp.maximum(w, 1.0)), -jnp.inf)
        g = jax.random.gumbel(key, (n,))
        bn = jnp.sort(jax.lax.top_k(logw + g, BATCH)[1])
        return np.asarray(bn, dtype=np.int32)


def _decompose(edges, edge_times, edge_states, bin_bounds):
    n, B, bins = N_NODES, BATCH, N_BINS
    edges = np.asarray(edges)
    edge_times = np.asarray(edge_times, dtype=np.float32)
    edge_states = np.asarray(edge_states)
    bin_bounds = np.asarray(bin_bounds, dtype=np.float32)

    bn = _batch_nodes_host()

    ti, tj = np.triu_indices(B, k=1)
    i_table = bn[ti].astype(np.int64)
    j_table = bn[tj].astype(np.int64)
    P = i_table.size

    # the device computes border times as x * step in f32; verify exact
    step = np.float32(bin_bounds[1])
    ks = np.arange(bins + 1, dtype=np.float32)
    assert np.array_equal((ks * step).astype(np.float32), bin_bounds[: bins + 1]), \
        "bin_bounds is not an exact arithmetic progression in f32"
    assert float(bin_bounds[0]) == 0.0

    pos_in_batch = np.full(n, -1, np.int64)
    pos_in_batch[bn] = np.arange(B)
    e0 = np.asarray(edges[0], dtype=np.int64)
    e1 = np.asarray(edges[1], dtype=np.int64)
    u = pos_in_batch[e0]
    v = pos_in_batch[e1]
    selmask = (u >= 0) & (v >= 0)
    su, sv = u[selmask], v[selmask]
    st = edge_times[selmask]
    ss = edge_states[selmask].astype(np.uint8)
    r = su * (B - 1) - (su * (su - 1)) // 2 + (sv - su - 1)

    order = np.lexsort((ss, st, r))
    r, st, ss = r[order], st[order], ss[order]
    Esel = r.size

    jm = np.searchsorted(bin_bounds[:bins], st, side="right")

    pairs_per_core = P // N_CORES
    assert P % N_CORES == 0 and pairs_per_core % PAIRS_PER_ROW == 0
    rows_per_core = pairs_per_core // PAIRS_PER_ROW + 1  # 512 (last is pad)
    nrows = rows_per_core * N_CORES

    core = r // pairs_per_core
    r_in_core = r % pairs_per_core
    row = core * rows_per_core + r_in_core // PAIRS_PER_ROW

    border_off = 50 * (r_in_core % PAIRS_PER_ROW) + jm
    _, row_start_idx, row_counts = np.unique(row, return_index=True, return_counts=True)
    rank_in_row = np.arange(Esel) - np.repeat(row_start_idx, row_counts)
    epos = (border_off + rank_in_row).astype(np.int64)

    e_per_row = np.zeros(nrows, np.int64)
    np.add.at(e_per_row, row, 1)
    D = int(e_per_row.max()) if Esel else 0
    assert D < 50

    WP = W_BORDER + D + 1  # padded row width (incl. phantom 0.0 slot)
    WP += WP % 2  # keep even

    # dense per-slot edge arrays (the O(E) edge data in expanded form)
    INDARR = np.zeros((nrows, WP), np.uint8)
    ETARR = np.zeros((nrows, WP), np.float32)
    ESARR = np.zeros((nrows, WP), np.uint8)
    INDARR[row, epos] = 1
    ETARR[row, epos] = st
    ESARR[row, epos] = ss

    npairs_row = np.full(nrows, PAIRS_PER_ROW, np.int64)
    npairs_row[rows_per_core - 1 :: rows_per_core] = 0
    lam = npairs_row * 50 + e_per_row

    # packed pair table (i << 16 | j), one entry per pair
    NTS = 33
    TSP = np.zeros((nrows, NTS), np.int32)
    valid = npairs_row > 0
    row_ids = np.arange(nrows)
    first_pair = (row_ids // rows_per_core) * pairs_per_core + (
        row_ids % rows_per_core
    ) * PAIRS_PER_ROW
    idx = np.minimum(first_pair[valid, None] + np.arange(32)[None, :], P - 1)
    TSP[valid, :32] = ((i_table[idx] << 16) | j_table[idx]).astype(np.int32)

    return dict(
        bn=bn, D=D, WP=WP, NTS=NTS,
        nrows=nrows, rows_per_core=rows_per_core,
        INDARR=INDARR, ETARR=ETARR, ESARR=ESARR, TSP=TSP, lam=lam,
        step=float(step), bb_last=float(bin_bounds[bins]),
        L=int(lam.sum()),
    )


# ---------------------------------------------------------------------------
# device kernel
# ---------------------------------------------------------------------------

_KERNEL_CACHE = {}


def _patch_tile_drain():
    """This walrus build rejects >1 sync wait per instruction; split waits
    onto preceding NoOps (regular ops) / extra drains (final drain)."""
    import concourse.mybir as mybir
    from concourse import tile
    from concourse.vector_clock import ScopedClock

    if getattr(tile.TileContext, "_drain_patched", False):
        return

    def _drain_and_barrier(self, tick_clock, wait_clock):
        drain_inst = self.nc.sync.drain()
        wait_clock.add_sem_waits(
            drain_inst.ins, ScopedClock({None: tick_clock.global_clock})
        )
        si = drain_inst.ins.sync_info
        waits = list(si.on_wait or [])
        if len(waits) > 1:
            si.on_wait = waits[:1]
            for i in range(1, len(waits)):
                d2 = self.nc.sync.drain()
                d2.ins.sync_info = mybir.SyncInfo(
                    on_wait=waits[i : i + 1], on_update=[]
                )
        self.nc.all_engine_barrier()
        assert self.sems is not None
        popped = self.nc._tile_sem_poison_stack.pop()
        assert popped is self._sem_poison
        self.nc.clear_and_free_semaphores(list(self.sems.allocated().values()))
        self.nc.all_engine_barrier()

    tile.TileContext._drain_and_barrier = _drain_and_barrier

    _orig_commit = tile.TileContext._commit_instruction
    _ctr = [0]

    def _commit_split(self, inst, lazy_reg_writes=True):
        si = inst.sync_info
        if si is not None and si.on_wait and len(si.on_wait) > 1:
            waits = list(si.on_wait)
            for w in waits[:-1]:
                _ctr[0] += 1
                nop = mybir.InstNoOp(
                    name=f"waitsplit-{_ctr[0]}",
                    sync_info=mybir.SyncInfo(on_wait=[w], on_update=[]),
                    bass_nofuse=True,
                    engine=inst.engine,
                )
                _orig_commit(self, nop, lazy_reg_writes=False)
            si.on_wait = waits[-1:]
        return _orig_commit(self, inst, lazy_reg_writes)

    tile.TileContext._commit_instruction = _commit_split
    tile.TileContext._drain_patched = True


def _build_bass(D, WP, NTS, step, bb_last, rows_per_core):
    import concourse.bass as bass
    import concourse.mybir as mybir
    from concourse import tile

    _patch_tile_drain()

    f32 = mybir.dt.float32
    i16 = mybir.dt.int16
    i32 = mybir.dt.int32
    u8 = mybir.dt.uint8
    Alu = mybir.AluOpType

    NT = rows_per_core // 128  # tiles per core (4)
    BL = NTS * 50  # expanded base length (1650)

    nc = bass.Bass("TRN2", target_bir_lowering=False, debug=False,
                   num_devices=N_CORES)

    INDt = nc.dram_tensor("INDARR", [NT, 128, WP], u8, kind="ExternalInput")
    ETAt = nc.dram_tensor("ETARR", [NT, 128, WP], f32, kind="ExternalInput")
    ESAt = nc.dram_tensor("ESARR", [NT, 128, WP], u8, kind="ExternalInput")
    TSPt = nc.dram_tensor("TSP", [NT, 128, NTS], i32, kind="ExternalInput")

    OT = nc.dram_tensor("OT", [NT, 128, WP], f32, kind="ExternalOutput")
    ODT = nc.dram_tensor("ODT", [NT, 128, WP], f32, kind="ExternalOutput")
    OST = nc.dram_tensor("OST", [NT, 128, WP], i32, kind="ExternalOutput")
    OIE = nc.dram_tensor("OIE", [NT, 128, WP], u8, kind="ExternalOutput")
    OPK = nc.dram_tensor("OPK", [NT, 128, WP], i32, kind="ExternalOutput")

    with tile.TileContext(nc) as tc:
        with (
            tc.tile_pool(name="static", bufs=1) as sp,
            tc.tile_pool(name="inp", bufs=2) as ip,
            tc.tile_pool(name="work", bufs=2) as wpool,
            tc.tile_pool(name="out", bufs=2) as op,
        ):
            # static: w mod 50 pattern as int16
            wmod = sp.tile([128, ((WP + 49) // 50) * 50], i16)
            nc.gpsimd.iota(wmod[:], pattern=[[0, (WP + 49) // 50], [1, 50]],
                           base=0, channel_multiplier=0)

            for t in range(NT):
                indu8 = ip.tile([128, WP], u8, tag="indu8")
                nc.sync.dma_start(indu8[:], INDt[t])
                eta = ip.tile([128, WP], f32, tag="eta")
                nc.sync.dma_start(eta[:], ETAt[t])
                esa = ip.tile([128, WP], u8, tag="esa")
                nc.sync.dma_start(esa[:], ESAt[t])
                tsp = ip.tile([128, NTS], i32, tag="tsp")
                nc.sync.dma_start(tsp[:], TSPt[t])

                # --- insertion indicator & running count ---
                ind = wpool.tile([128, WP], i16, tag="ind")
                nc.vector.tensor_scalar(ind[:], indu8[:], 0.0, None,
                                        op0=Alu.is_gt)
                nins = wpool.tile([128, WP], i16, tag="nins")
                nc.vector.tensor_tensor_scan(
                    nins[:], ind[:], ind[:], 0.0, op0=Alu.add, op1=Alu.bypass
                )

                # --- x = (w - nins) mod 50, in-pair border index ---
                t1 = wpool.tile([128, WP], i16, tag="t1")
                nc.vector.tensor_tensor(t1[:], wmod[:, 0:WP], nins[:],
                                        op=Alu.subtract)
                c150 = wpool.tile([128, WP], i16, tag="c150")
                nc.vector.tensor_scalar(c150[:], t1[:], 0.0, 50.0,
                                        op0=Alu.is_lt, op1=Alu.mult)
                x = wpool.tile([128, WP], i16, tag="x")
                nc.vector.tensor_tensor(x[:], t1[:], c150[:], op=Alu.add)

                # --- times = x * step, edge times injected ---
                times = op.tile([128, WP], f32, tag="times")
                nc.scalar.mul(times[:], x[:], step)
                nc.vector.copy_predicated(times[:], ind[:], eta[:])

                # --- packed pair ids: select shifted views of expanded table ---
                base = wpool.tile([128, 50 + BL], i32, tag="base")
                nc.vector.tensor_copy(
                    base[:, 50 : 50 + BL], tsp[:].broadcast_to([128, NTS, 50])
                )
                pk = op.tile([128, WP], i32, tag="pk")
                nc.scalar.copy(pk[:], base[:, 50 : 50 + WP])
                nc.vector.copy_predicated(pk[:], c150[:], base[:, 0:WP])

                # --- expanded states: forward-fill scan ---
                k16 = wpool.tile([128, WP], i16, tag="k16")
                nc.vector.tensor_scalar(k16[:], x[:], 0.0, None,
                                        op0=Alu.not_equal)
                keepf = wpool.tile([128, WP], f32, tag="keepf")
                nc.vector.tensor_tensor(keepf[:], k16[:], ind[:], op=Alu.is_gt)
                esf = wpool.tile([128, WP], f32, tag="esf")
                nc.scalar.copy(esf[:], esa[:])
                est = op.tile([128, WP], f32, tag="est")
                nc.vector.tensor_tensor_scan(
                    est[:], keepf[:], esf[:], 0.0, op0=Alu.mult, op1=Alu.add
                )

                # --- dt: shifted diff + wrap ---
                dt = op.tile([128, WP], f32, tag="dt")
                nc.vector.tensor_tensor(
                    dt[:, 0 : WP - 1], times[:, 1:WP], times[:, 0 : WP - 1],
                    op=Alu.subtract,
                )
                dneg = wpool.tile([128, WP], f32, tag="dneg")
                nc.vector.tensor_scalar(dneg[:], dt[:], 0.0, bb_last,
                                        op0=Alu.is_lt, op1=Alu.mult)
                nc.vector.tensor_tensor(dt[:], dt[:], dneg[:], op=Alu.add)

                # --- stores ---
                nc.sync.dma_start(OT[t], times[:])
                nc.sync.dma_start(ODT[t], dt[:])
                nc.sync.dma_start(OPK[t], pk[:])
                nc.gpsimd.dma_start(OST[t], est[:])   # f32 -> i32 cast
                nc.gpsimd.dma_start(OIE[t], ind[:])   # i16 -> u8 cast

    return nc


def _get_bass(meta):
    key = (meta["D"], meta["WP"], meta["NTS"], meta["step"], meta["bb_last"],
           meta["rows_per_core"])
    if key not in _KERNEL_CACHE:
        _KERNEL_CACHE[key] = _build_bass(*key)
    return _KERNEL_CACHE[key]


def _run_device(meta, trace=False):
    from concourse.bass_utils import run_bass_kernel_spmd

    nc = _get_bass(meta)
    NT = meta["rows_per_core"] // 128
    WP = meta["WP"]
    rpc = meta["rows_per_core"]

    in_maps = []
    for c in range(N_CORES):
        sl = slice(c * rpc, (c + 1) * rpc)
        in_maps.append({
            "INDARR": meta["INDARR"][sl].reshape(NT, 128, WP),
            "ETARR": meta["ETARR"][sl].reshape(NT, 128, WP),
            "ESARR": meta["ESARR"][sl].reshape(NT, 128, WP),
            "TSP": meta["TSP"][sl].reshape(NT, 128, meta["NTS"]),
        })

    return run_bass_kernel_spmd(
        nc, in_maps, core_ids=list(range(N_CORES)), trace=trace
    )


def _assemble(meta, res):
    nrows = meta["nrows"]
    WP = meta["WP"]
    lam = meta["lam"]
    L = meta["L"]

    def stack(name):
        return np.concatenate(
            [res.results[c][name].reshape(-1, WP) for c in range(N_CORES)], axis=0
        )

    row_of = np.repeat(np.arange(nrows), lam)
    starts = np.concatenate([[0], np.cumsum(lam)[:-1]])
    w_of = np.arange(L) - np.repeat(starts, lam)
    src = row_of * WP + w_of

    times = stack("OT").reshape(-1)[src]
    dt = stack("ODT").reshape(-1)[src]
    est = stack("OST").reshape(-1)[src]
    ie = stack("OIE").reshape(-1)[src].astype(bool)
    pk = stack("OPK").reshape(-1)[src]

    i_out = (pk >> 16).astype(np.int32)
    j_out = (pk & 0xFFFF).astype(np.int32)
    pairs = np.stack([i_out, j_out])
    return (meta["bn"], pairs, times.astype(np.float32),
            est.astype(np.int32), ie, dt.astype(np.float32))


def kernel(edges, edge_times, edge_states, bin_bounds, nodes_num, batch_size,
           _trace=False, _return_res=False):
    meta = _decompose(edges, edge_times, edge_states, bin_bounds)
    res = _run_device(meta, trace=_trace)
    out = _assemble(meta, res)
    if _return_res:
        return out, res, meta
    return out
